# revision 22
# baseline (speedup 1.0000x reference)
"""Distributed multi-head attention for TRN2, 8 NeuronCores.

Sharding: tensor-parallel over heads (2 heads / core) for QKV + attention;
an AllToAll exchanges normalized attention outputs so each core computes
the output projection for its own 512 sequence rows.

Perf structure (v3):
- input x streamed piece-major (8 seq-pieces x all dim-chunks, 4KB/
  partition lines) on BOTH hardware DGE queues (sync + scalar).
- scores emitted as (h0, h1) pairs -> disjoint PE quadrants run them
  concurrently; macros of 2 key-tiles keep same-shape matmul chains
  long so LDWEIGHTS stays hidden.
- PV matmuls lag scores by LAG groups: exp latency (either engine)
  never gates the PE.
- exp split across ScalarE (exact LUT) and VectorE (Schraudolph
  bit-trick: i16 = round(s*A + B) reinterpreted as bf16) with
  credit-based balancing.
- dummy matmuls keep the PE clock warm across the AllToAll wait.
"""
import numpy as np
import ml_dtypes

import concourse.bass as bass
import concourse.tile as tile
from concourse import bacc, mybir
from concourse.bass_utils import run_bass_kernel_spmd

# problem dims (hardcoded; kernel.py must be self-contained)
N, DIM, HEADS, DH = 4096, 1024, 16, 64
NCORES = 8
HPC = HEADS // NCORES        # 2 heads per core
ICB = HPC * DH               # 128 inner dims per core
DCH = DIM // 128             # 8 dim chunks
QC = 512                     # query-chunk (columns per scores matmul)
NQ = N // QC                 # 8
KT = 128                     # key tile (scores output partitions)
NKT = N // KT                # 32
SEQC = N // NCORES           # 512 output rows per core
SCALE = float(DH) ** -0.5
LAG = 8                      # PV pipeline lag (groups)

# Schraudolph-exp constants for the DVE share: bf16 bits of exp(s*SCALE)
# ~= round(s * EXPA + EXPB); worst-case rel err ~3.3%, which the softmax
# normalization mostly cancels (measured end-to-end ~7e-3 absmax-rel).
EXPA = SCALE * 1.4426950408889634 * 128.0
EXPB = 16256.0 - 5.35

BF16 = mybir.dt.bfloat16
F32 = mybir.dt.float32
F8 = mybir.dt.float8e4
I16 = mybir.dt.int16
BF16_NP = ml_dtypes.bfloat16

# v layout: per-head block padded to 80 cols (65 used) so DoubleRow
# access-pattern strides are 16-aligned; t-stride 160
VHB = 80
# per-MACRO exp cost (ns) for ACT/DVE load balancing at trace time.
# macro = 2 key-tile groups: ACT macros get exact exp + fp8 DoubleRow PV,
# DVE macros get Schraudolph exp + bf16-rhs PV.
ACT_MACRO_NS = 2 * (1024 + 352) / 1.2
DVE_MACRO_NS = 2 * (1024 + 271) / 0.96
EPI_DVE_NS = 3400.0          # den copy + recip + an-mul per j on DVE


def build_kernel():
    nc = bacc.Bacc("TRN2", target_bir_lowering=False, debug=False,
                   enable_asserts=True, num_devices=NCORES)

    # x piece-major: [128 dim-rows, piece, dim-chunk, 512 seq]
    xt = nc.dram_tensor("xt", [128, NQ, DCH, QC], BF16, kind="ExternalInput")
    wq = nc.dram_tensor("wq", [128, DCH, ICB], BF16, kind="ExternalInput")
    wk = nc.dram_tensor("wk", [128, DCH, ICB], BF16, kind="ExternalInput")
    wv = nc.dram_tensor("wv", [128, DCH, ICB], BF16, kind="ExternalInput")
    wo = nc.dram_tensor("wo", [128, DCH, DIM], BF16, kind="ExternalInput")
    bo = nc.dram_tensor("bo", [128, DIM], F32, kind="ExternalInput")
    out = nc.dram_tensor("out", [SEQC, DIM], F32, kind="ExternalOutput")
    wsink = nc.dram_tensor("warm_sink", [128, 16], F32, kind="ExternalOutput")

    with tile.TileContext(nc) as tc:
        with (
            tc.tile_pool(name="xtp", bufs=NQ) as xtp,
            tc.tile_pool(name="wp", bufs=1) as wp,
            tc.tile_pool(name="qk", bufs=1) as qkp,
            tc.tile_pool(name="dram", bufs=1, space="DRAM") as dramp,
        ):
            # ---- load inputs: weights first (k-proj gate), then x pieces
            # split across BOTH hardware DGE queues ----
            wq_t = wp.tile([128, DCH, ICB], BF16, tag="wq")
            wk_t = wp.tile([128, DCH, ICB], BF16, tag="wk")
            wv_t = wp.tile([128, DCH, ICB], BF16, tag="wv")
            wo_t = wp.tile([128, DCH, DIM], BF16, tag="wo")
            bo_t = wp.tile([128, DIM], F32, tag="bo")
            nc.sync.dma_start(wk_t[:], wk[:])
            nc.scalar.dma_start(wq_t[:], wq[:])
            nc.scalar.dma_start(wv_t[:], wv[:])
            xt_t = [xtp.tile([128, DCH, QC], BF16, tag="xt", name=f"xt{p}")
                    for p in range(NQ)]
            for p in range(NQ):
                half = DCH // 2
                nc.sync.dma_start(xt_t[p][:, 0:half, :], xt[:, p, 0:half, :])
                nc.scalar.dma_start(xt_t[p][:, half:DCH, :],
                                    xt[:, p, half:DCH, :])
            # wo/bo on the sync queue only: the scalar engine's DMA issues
            # serialize with transfer completions, and anything queued there
            # would delay the first exps (in-order engine queue)
            for d in range(DCH):
                nc.sync.dma_start(wo_t[:, d, :], wo[:, d, :])
            nc.sync.dma_start(bo_t[:], bo[:])

            # early barrier: absorb inter-core startup skew during the ramp
            bar_i = dramp.tile([1, 16], F32, tag="bar_i")
            bar_o = dramp.tile([1, 16], F32, tag="bar_o", addr_space="Shared")
            nc.gpsimd.dma_start(bar_i[:], wk[0:1, 0, 0:16])
            nc.gpsimd.collective_compute(
                "AllReduce", mybir.AluOpType.add,
                replica_groups=[list(range(NCORES))],
                ins=[bar_i.opt()], outs=[bar_o.opt()],
            )

            qT = qkp.tile([128, N], BF16, tag="qT")   # [2 heads x 64, seq]
            kT = qkp.tile([128, N], BF16, tag="kT")
            # v natural layout + ones column per head, fp8 (PV runs fp8;
            # exact for the DoubleRow share, bf16-rhs for the rest)
            vt = qkp.tile([128, NKT, HPC * VHB], F8, tag="vt")
            nc.gpsimd.memset(vt[:], 1.0)

            a2a_in = dramp.tile([NCORES, ICB, QC], BF16, tag="a2a_in")
            a2a_out = dramp.tile([NCORES, ICB, QC], BF16, tag="a2a_out")

            wz = wp.tile([128, QC], BF16, tag="wz")
            nc.gpsimd.memset(wz[:], 0.0)

            # ---- projections per piece: k block, q block, v tiles ----
            with tc.tile_pool(name="psA", bufs=4, space="PSUM") as psA:
                # warm-up: dep-free matmuls while the first DMAs stream
                w_ps = psA.tile([128, QC], F32, tag="warm", name="warm_ps")
                last_warm = None
                for _ in range(32):
                    last_warm = nc.tensor.matmul(w_ps[:], wz[:, 0:128], wz[:],
                                                 start=True, stop=True)
                wcp = wp.tile([128, 16], F32, tag="wcp")
                nc.vector.tensor_copy(wcp[:], w_ps[:, 0:16])
                nc.sync.dma_start(wsink[:], wcp[:])

                first_real = None
                for p in range(NQ):
                    for dst, w_t in ((kT, wk_t), (qT, wq_t)):
                        ps = psA.tile([128, QC], F32, tag="proj",
                                      name=f"pj{p}_{0 if dst is kT else 1}")
                        for d in range(DCH):
                            m = nc.tensor.matmul(
                                ps[:], w_t[:, d, :], xt_t[p][:, d, :],
                                start=(d == 0), stop=(d == DCH - 1))
                            if first_real is None:
                                first_real = m
                        # DVE copy: ScalarE's queue is busy issuing piece DMAs
                        # (in-order engine; a copy there stalls the proj ring)
                        nc.vector.tensor_copy(dst[:, p * QC:(p + 1) * QC], ps[:])
                    # v tiles for this piece (natural layout)
                    vs = psA.tile([128, QC], F32, tag="proj", name=f"pv{p}")
                    vs4 = vs[:].rearrange("p (t k) -> p t k", t=4)
                    for tt in range(4):
                        for d in range(DCH):
                            nc.tensor.matmul(
                                vs4[:, tt, :],
                                xt_t[p][:, d, tt * KT:(tt + 1) * KT],
                                wv_t[:, d, :],
                                start=(d == 0), stop=(d == DCH - 1))
                    for tt in range(4):
                        t = 4 * p + tt
                        nc.vector.tensor_copy(vt[:, t, 0:DH], vs4[:, tt, 0:DH])
                        nc.vector.tensor_copy(vt[:, t, VHB:VHB + DH],
                                              vs4[:, tt, DH:ICB])
                bass._add_dep_helper(first_real.ins, last_warm.ins, sync=False,
                                     reason="warm-up runs before projections")

            with (
                tc.tile_pool(name="psS", bufs=3, space="PSUM") as psS,
                tc.tile_pool(name="psV", bufs=1, space="PSUM") as psV,
                tc.tile_pool(name="expp", bufs=14) as expp,
                tc.tile_pool(name="expf8", bufs=7) as expf8,
                tc.tile_pool(name="attp", bufs=2) as attp,
                tc.tile_pool(name="invp", bufs=2) as invp,
                tc.tile_pool(name="invL", bufs=1) as invL,
            ):
                # ---- attention: macros of 2 key-tiles; PV lags LAGM macros
                # behind scores/exp so exp latency never gates the PE ----
                slots = [(j, t) for j in range(NQ) for t in range(NKT)]
                NM = len(slots) // 2        # macros; never straddle j
                LAGM = LAG // 2
                pv = {}
                ex_of = {}
                load = {"act": 0.0, "dve": 0.0}

                def emit_scores(s):
                    j, t = slots[s]
                    sc = psS.tile([128, HPC, QC], F32, tag="sc")
                    for h in range(HPC):
                        nc.tensor.matmul(
                            sc[:, h, :],
                            kT[h * DH:(h + 1) * DH, t * KT:(t + 1) * KT],
                            qT[h * DH:(h + 1) * DH, j * QC:(j + 1) * QC],
                            start=True, stop=True,
                        )
                    return sc

                def emit_exp(m, sc0, sc1):
                    if load["act"] <= load["dve"]:
                        # exact exp -> fp8, consumed by a DoubleRow PV
                        ex = expf8.tile([128, 2, HPC, QC], F8, tag="ex8")
                        for i, sc in enumerate((sc0, sc1)):
                            nc.scalar.activation(
                                ex[:, i, :, :], sc[:],
                                mybir.ActivationFunctionType.Exp, scale=SCALE)
                        load["act"] += ACT_MACRO_NS
                        ex_of[m] = ("dr", ex)
                    else:
                        # Schraudolph bit-trick exp -> bf16, normal PVs
                        exs = []
                        for i, sc in enumerate((sc0, sc1)):
                            ex = expp.tile([128, HPC, QC], BF16, tag="ex",
                                           name=f"ex{m}_{i}")
                            nc.vector.tensor_scalar(
                                ex[:].bitcast(I16), sc[:], EXPA, EXPB,
                                mybir.AluOpType.mult, mybir.AluOpType.add)
                            exs.append(ex)
                        load["dve"] += DVE_MACRO_NS
                        ex_of[m] = ("nrm", exs)

                def emit_pv(m):
                    j, t0 = slots[2 * m]
                    if j not in pv:
                        pv[j] = psV.tile([128, HPC, QC], F32, tag="pv",
                                         name=f"pv{j}")
                    kind, ex = ex_of.pop(m)
                    if kind == "dr":
                        for h in range(HPC):
                            nc.tensor.matmul(
                                pv[j][0:DH + 1, h, :],
                                vt[:, t0:t0 + 2, h * VHB:h * VHB + DH + 1],
                                ex[:, :, h, :],
                                start=(t0 == 0), stop=(t0 + 1 == NKT - 1),
                                perf_mode=mybir.MatmulPerfMode.DoubleRow,
                            )
                    else:
                        for i in range(2):
                            for h in range(HPC):
                                nc.tensor.matmul(
                                    pv[j][0:DH + 1, h, :],
                                    vt[:, t0 + i, h * VHB:h * VHB + DH + 1],
                                    ex[i][:, h, :],
                                    start=(t0 + i == 0),
                                    stop=(t0 + i == NKT - 1),
                                )
                    if t0 + 1 == NKT - 1:
                        emit_epilogue(j)

                def emit_epilogue(j):
                    if j == NQ - 1:
                        # latency-optimized per-head pipeline for the last j
                        den = [invL.tile([1, QC], F32, tag=f"denL{h}",
                                         name=f"denL{h}") for h in range(HPC)]
                        inv = [invL.tile([1, QC], F32, tag=f"invL{h}",
                                         name=f"invL{h}") for h in range(HPC)]
                        invb = [invL.tile([DH, QC], F32, tag=f"invbL{h}",
                                          name=f"invbL{h}") for h in range(HPC)]
                        an = [invL.tile([DH, QC], BF16, tag=f"anL{h}",
                                        name=f"anL{h}") for h in range(HPC)]
                        nc.vector.tensor_copy(den[0][:], pv[j][DH:DH + 1, 0, :])
                        nc.vector.reciprocal_approx_fast(inv[0][:], den[0][:])
                        nc.gpsimd.partition_broadcast(invb[0][:], inv[0][:])
                        nc.vector.tensor_copy(den[1][:], pv[j][DH:DH + 1, 1, :])
                        nc.vector.reciprocal_approx_fast(inv[1][:], den[1][:])
                        nc.vector.tensor_mul(an[0][:], pv[j][0:DH, 0, :],
                                             invb[0][:])
                        nc.gpsimd.partition_broadcast(invb[1][:], inv[1][:])
                        nc.sync.dma_start(a2a_in[j, 0:DH, :], an[0][:])
                        nc.vector.tensor_mul(an[1][:], pv[j][0:DH, 1, :],
                                             invb[1][:])
                        nc.sync.dma_start(a2a_in[j, DH:ICB, :], an[1][:])
                    else:
                        den = invp.tile([1, HPC, QC], F32, tag="den",
                                        name=f"den{j}")
                        inv = invp.tile([1, HPC, QC], F32, tag="inv",
                                        name=f"inv{j}")
                        invb = invp.tile([DH, HPC, QC], F32, tag="invb",
                                         name=f"invb{j}")
                        an = attp.tile([DH, HPC, QC], BF16, tag="an",
                                       name=f"an{j}")
                        nc.vector.tensor_copy(den[:], pv[j][DH:DH + 1, :, :])
                        nc.vector.reciprocal_approx_fast(inv[:], den[:])
                        for h in range(HPC):
                            nc.gpsimd.partition_broadcast(invb[:, h, :],
                                                          inv[:, h, :])
                        nc.vector.tensor_mul(an[:], pv[j][0:DH, :, :], invb[:])
                        load["dve"] += EPI_DVE_NS
                        for h in range(HPC):
                            nc.sync.dma_start(a2a_in[j, h * DH:(h + 1) * DH, :],
                                              an[:, h, :])
                    del pv[j]
                    if j == NQ - 2:
                        # re-sync cores while the last q-chunk computes
                        bar2_i = dramp.tile([1, 16], F32, tag="bar2_i")
                        bar2_o = dramp.tile([1, 16], F32, tag="bar2_o",
                                            addr_space="Shared")
                        nc.gpsimd.dma_start(bar2_i[:], a2a_in[j, 0:1, 0:16])
                        nc.gpsimd.collective_compute(
                            "AllReduce", mybir.AluOpType.add,
                            replica_groups=[list(range(NCORES))],
                            ins=[bar2_i.opt()], outs=[bar2_o.opt()],
                        )

                # macro loop: 2 score-groups, 2 exps, then a lagged PV macro
                for m in range(NM):
                    sc0 = emit_scores(2 * m)
                    sc1 = emit_scores(2 * m + 1)
                    emit_exp(m, sc0, sc1)
                    if m - LAGM >= 0:
                        emit_pv(m - LAGM)
                for m in range(NM - LAGM, NM):
                    emit_pv(m)

            # ---- exchange: my (2 heads x all seq) -> (all inner x my seq) ----
            nc.gpsimd.collective_compute(
                "AllToAll", mybir.AluOpType.bypass,
                replica_groups=[list(range(NCORES))],
                ins=[a2a_in.opt()], outs=[a2a_out.opt()],
            )

            # ---- output projection for my SEQC rows ----
            with (
                tc.tile_pool(name="psC", bufs=2, space="PSUM") as psC,
                tc.tile_pool(name="psW", bufs=1, space="PSUM") as psW,
                tc.tile_pool(name="afp", bufs=1) as afp,
                tc.tile_pool(name="finp", bufs=2) as finp,
            ):
                # keep the PE warm through the AllToAll wait
                w2_ps = psW.tile([128, QC], F32, tag="warm2")
                for _ in range(150):
                    nc.tensor.matmul(w2_ps[:], wz[:, 0:128], wz[:],
                                     start=True, stop=True)
                af = afp.tile([128, NCORES, QC], BF16, tag="af")
                for r in range(NCORES):
                    (nc.sync if r % 2 == 0 else nc.scalar).dma_start(
                        af[:, r, :], a2a_out[r])
                bo3 = bo_t[:].rearrange("p (a b) -> p a b", a=2)
                for s in range(SEQC // 128):
                    yps = psC.tile([128, 2, QC], F32, tag="y")
                    for r in range(NCORES):
                        for half in range(2):
                            nc.tensor.matmul(
                                yps[:, half, :],
                                af[:, r, s * 128:(s + 1) * 128],
                                wo_t[:, r, half * QC:(half + 1) * QC],
                                start=(r == 0), stop=(r == NCORES - 1))
                    ysb = finp.tile([128, 2, QC], F32, tag="ysb")
                    nc.vector.tensor_add(ysb[:], yps[:], bo3)
                    orows = out[s * 128:(s + 1) * 128, :].rearrange(
                        "p (a b) -> p a b", a=2)
                    for half in range(2):
                        (nc.sync if half == 0 else nc.scalar).dma_start(
                            orows[:, half, :], ysb[:, half, :])

    nc.compile()
    return nc


_NC_CACHE = None


def _get_nc():
    global _NC_CACHE
    if _NC_CACHE is None:
        _NC_CACHE = build_kernel()
    return _NC_CACHE


def _prep_inputs(x, Wq, Wk, Wv, Wo, bo):
    """Host-side sharding/layout prep (untimed)."""
    # [128 dim-rows, piece, dim-chunk, 512 seq-cols]
    xt_p = np.ascontiguousarray(
        x.T.reshape(DCH, 128, NQ, QC).transpose(1, 2, 0, 3)).astype(BF16_NP)
    wo_p = np.ascontiguousarray(
        Wo.reshape(DCH, 128, DIM).transpose(1, 0, 2)).astype(BF16_NP)
    bo_p = np.ascontiguousarray(np.tile(bo[None, :], (128, 1))).astype(np.float32)
    in_maps = []
    for c in range(NCORES):
        ic = slice(c * ICB, (c + 1) * ICB)
        m = {"xt": xt_p, "wo": wo_p, "bo": bo_p}
        for name, W in (("wq", Wq), ("wk", Wk), ("wv", Wv)):
            m[name] = np.ascontiguousarray(
                W[:, ic].reshape(DCH, 128, ICB).transpose(1, 0, 2)).astype(BF16_NP)
        in_maps.append(m)
    return in_maps


def kernel(x, Wq, Wk, Wv, Wo, bo, _trace=False):
    x = np.asarray(x, np.float32)
    Wq = np.asarray(Wq, np.float32)
    Wk = np.asarray(Wk, np.float32)
    Wv = np.asarray(Wv, np.float32)
    Wo = np.asarray(Wo, np.float32)
    bo = np.asarray(bo, np.float32)
    nc = _get_nc()
    in_maps = _prep_inputs(x, Wq, Wk, Wv, Wo, bo)
    r = run_bass_kernel_spmd(nc, in_maps, core_ids=list(range(NCORES)),
                             trace=_trace)
    y = np.concatenate([r.results[c]["out"] for c in range(NCORES)], axis=0)
    if _trace:
        kernel.last_result = r
    return y.astype(np.float32)


# revision 24
# speedup vs baseline: 1.0140x; 1.0140x over previous
"""Distributed multi-head attention for TRN2, 8 NeuronCores.

Sharding: tensor-parallel over heads (2 heads / core) for QKV + attention;
an AllToAll exchanges normalized attention outputs so each core computes
the output projection for its own 512 sequence rows.

Perf structure (v3):
- input x streamed piece-major (8 seq-pieces x all dim-chunks, 4KB/
  partition lines) on BOTH hardware DGE queues (sync + scalar).
- scores emitted as (h0, h1) pairs -> disjoint PE quadrants run them
  concurrently; macros of 2 key-tiles keep same-shape matmul chains
  long so LDWEIGHTS stays hidden.
- PV matmuls lag scores by LAG groups: exp latency (either engine)
  never gates the PE.
- exp split across ScalarE (exact LUT) and VectorE (Schraudolph
  bit-trick: i16 = round(s*A + B) reinterpreted as bf16) with
  credit-based balancing.
- dummy matmuls keep the PE clock warm across the AllToAll wait.
"""
import numpy as np
import ml_dtypes

import concourse.bass as bass
import concourse.tile as tile
from concourse import bacc, mybir
from concourse.bass_utils import run_bass_kernel_spmd

# problem dims (hardcoded; kernel.py must be self-contained)
N, DIM, HEADS, DH = 4096, 1024, 16, 64
NCORES = 8
HPC = HEADS // NCORES        # 2 heads per core
ICB = HPC * DH               # 128 inner dims per core
DCH = DIM // 128             # 8 dim chunks
QC = 512                     # query-chunk (columns per scores matmul)
NQ = N // QC                 # 8
KT = 128                     # key tile (scores output partitions)
NKT = N // KT                # 32
SEQC = N // NCORES           # 512 output rows per core
SCALE = float(DH) ** -0.5
LAG = 8                      # PV pipeline lag (groups)

# Schraudolph-exp constants for the DVE share: bf16 bits of exp(s*SCALE)
# ~= round(s * EXPA + EXPB); worst-case rel err ~3.3%, which the softmax
# normalization mostly cancels (measured end-to-end ~7e-3 absmax-rel).
EXPA = SCALE * 1.4426950408889634 * 128.0
EXPB = 16256.0 - 5.35

BF16 = mybir.dt.bfloat16
F32 = mybir.dt.float32
F8 = mybir.dt.float8e4
I16 = mybir.dt.int16
BF16_NP = ml_dtypes.bfloat16

# v layout: per-head block padded to 80 cols (65 used) so DoubleRow
# access-pattern strides are 16-aligned; t-stride 160
VHB = 80
# per-MACRO exp cost (ns) for ACT/DVE load balancing at trace time.
# macro = 2 key-tile groups: ACT macros get exact exp + fp8 DoubleRow PV,
# DVE macros get Schraudolph exp + bf16-rhs PV.
ACT_MACRO_NS = 2 * (1024 + 352) / 1.2
DVE_MACRO_NS = 2 * (1024 + 271) / 0.96
EPI_DVE_NS = 3400.0          # den copy + recip + an-mul per j on DVE


def build_kernel():
    nc = bacc.Bacc("TRN2", target_bir_lowering=False, debug=False,
                   enable_asserts=True, num_devices=NCORES)

    # x piece-major: [128 dim-rows, piece, dim-chunk, 512 seq]
    xt = nc.dram_tensor("xt", [128, NQ, DCH, QC], BF16, kind="ExternalInput")
    wq = nc.dram_tensor("wq", [128, DCH, ICB], BF16, kind="ExternalInput")
    wk = nc.dram_tensor("wk", [128, DCH, ICB], BF16, kind="ExternalInput")
    wv = nc.dram_tensor("wv", [128, DCH, ICB], BF16, kind="ExternalInput")
    wo = nc.dram_tensor("wo", [128, DCH, DIM], BF16, kind="ExternalInput")
    bo = nc.dram_tensor("bo", [128, DIM], F32, kind="ExternalInput")
    out = nc.dram_tensor("out", [SEQC, DIM], F32, kind="ExternalOutput")
    wsink = nc.dram_tensor("warm_sink", [128, 16], F32, kind="ExternalOutput")

    with tile.TileContext(nc) as tc:
        with (
            tc.tile_pool(name="xtp", bufs=NQ) as xtp,
            tc.tile_pool(name="wp", bufs=1) as wp,
            tc.tile_pool(name="qk", bufs=1) as qkp,
            tc.tile_pool(name="dram", bufs=1, space="DRAM") as dramp,
        ):
            # ---- load inputs: weights first (k-proj gate), then x pieces
            # split across BOTH hardware DGE queues ----
            wq_t = wp.tile([128, DCH, ICB], BF16, tag="wq")
            wk_t = wp.tile([128, DCH, ICB], BF16, tag="wk")
            wv_t = wp.tile([128, DCH, ICB], BF16, tag="wv")
            wo_t = wp.tile([128, DCH, DIM], BF16, tag="wo")
            bo_t = wp.tile([128, DIM], F32, tag="bo")
            nc.sync.dma_start(wk_t[:], wk[:])
            nc.scalar.dma_start(wq_t[:], wq[:])
            nc.scalar.dma_start(wv_t[:], wv[:])
            xt_t = [xtp.tile([128, DCH, QC], BF16, tag="xt", name=f"xt{p}")
                    for p in range(NQ)]
            for p in range(NQ):
                half = DCH // 2
                nc.sync.dma_start(xt_t[p][:, 0:half, :], xt[:, p, 0:half, :])
                nc.scalar.dma_start(xt_t[p][:, half:DCH, :],
                                    xt[:, p, half:DCH, :])
            # wo/bo on the sync queue only: the scalar engine's DMA issues
            # serialize with transfer completions, and anything queued there
            # would delay the first exps (in-order engine queue)
            for d in range(DCH):
                nc.sync.dma_start(wo_t[:, d, :], wo[:, d, :])
            nc.sync.dma_start(bo_t[:], bo[:])

            # early barrier: absorb inter-core startup skew during the ramp
            bar_i = dramp.tile([1, 16], F32, tag="bar_i")
            bar_o = dramp.tile([1, 16], F32, tag="bar_o", addr_space="Shared")
            nc.gpsimd.dma_start(bar_i[:], wk[0:1, 0, 0:16])
            nc.gpsimd.collective_compute(
                "AllReduce", mybir.AluOpType.add,
                replica_groups=[list(range(NCORES))],
                ins=[bar_i.opt()], outs=[bar_o.opt()],
            )

            qT = qkp.tile([128, N], BF16, tag="qT")   # [2 heads x 64, seq]
            kT = qkp.tile([128, N], BF16, tag="kT")
            # v natural layout + ones column per head, fp8 (PV runs fp8;
            # exact for the DoubleRow share, bf16-rhs for the rest)
            vt = qkp.tile([128, NKT, HPC * VHB], F8, tag="vt")
            nc.gpsimd.memset(vt[:], 1.0)

            a2a_in = dramp.tile([NCORES, ICB, QC], BF16, tag="a2a_in")
            a2a_out = dramp.tile([NCORES, ICB, QC], BF16, tag="a2a_out")

            wz = wp.tile([128, QC], BF16, tag="wz")
            nc.gpsimd.memset(wz[:], 0.0)

            # ---- projections per piece: k block, q block, v tiles ----
            with tc.tile_pool(name="psA", bufs=4, space="PSUM") as psA:
                # warm-up: dep-free matmuls while the first DMAs stream
                w_ps = psA.tile([128, QC], F32, tag="warm", name="warm_ps")
                last_warm = None
                for _ in range(32):
                    last_warm = nc.tensor.matmul(w_ps[:], wz[:, 0:128], wz[:],
                                                 start=True, stop=True)
                wcp = wp.tile([128, 16], F32, tag="wcp")
                nc.vector.tensor_copy(wcp[:], w_ps[:, 0:16])
                nc.sync.dma_start(wsink[:], wcp[:])

                first_real = None
                for p in range(NQ):
                    for dst, w_t in ((kT, wk_t), (qT, wq_t)):
                        ps = psA.tile([128, QC], F32, tag="proj",
                                      name=f"pj{p}_{0 if dst is kT else 1}")
                        for d in range(DCH):
                            m = nc.tensor.matmul(
                                ps[:], w_t[:, d, :], xt_t[p][:, d, :],
                                start=(d == 0), stop=(d == DCH - 1))
                            if first_real is None:
                                first_real = m
                        # DVE copy: ScalarE's queue is busy issuing piece DMAs
                        # (in-order engine; a copy there stalls the proj ring)
                        nc.vector.tensor_copy(dst[:, p * QC:(p + 1) * QC], ps[:])
                    # v tiles for this piece (natural layout)
                    vs = psA.tile([128, QC], F32, tag="proj", name=f"pv{p}")
                    vs4 = vs[:].rearrange("p (t k) -> p t k", t=4)
                    for tt in range(4):
                        for d in range(DCH):
                            nc.tensor.matmul(
                                vs4[:, tt, :],
                                xt_t[p][:, d, tt * KT:(tt + 1) * KT],
                                wv_t[:, d, :],
                                start=(d == 0), stop=(d == DCH - 1))
                    for tt in range(4):
                        t = 4 * p + tt
                        nc.vector.tensor_copy(vt[:, t, 0:DH], vs4[:, tt, 0:DH])
                        nc.vector.tensor_copy(vt[:, t, VHB:VHB + DH],
                                              vs4[:, tt, DH:ICB])
                bass._add_dep_helper(first_real.ins, last_warm.ins, sync=False,
                                     reason="warm-up runs before projections")

            with (
                tc.tile_pool(name="psS", bufs=2, space="PSUM") as psS,
                tc.tile_pool(name="psV", bufs=2, space="PSUM") as psV,
                tc.tile_pool(name="expp", bufs=14) as expp,
                tc.tile_pool(name="expf8", bufs=7) as expf8,
                tc.tile_pool(name="attp", bufs=2) as attp,
                tc.tile_pool(name="invp", bufs=2) as invp,
                tc.tile_pool(name="invL", bufs=1) as invL,
            ):
                # ---- attention: macros of 2 key-tiles; PV lags LAGM macros
                # behind scores/exp so exp latency never gates the PE ----
                slots = [(j, t) for j in range(NQ) for t in range(NKT)]
                NM = len(slots) // 2        # macros; never straddle j
                LAGM = LAG // 2
                pv = {}
                ex_of = {}
                load = {"act": 0.0, "dve": 0.0}

                def emit_scores(s):
                    j, t = slots[s]
                    sc = psS.tile([128, HPC, QC], F32, tag="sc")
                    for h in range(HPC):
                        nc.tensor.matmul(
                            sc[:, h, :],
                            kT[h * DH:(h + 1) * DH, t * KT:(t + 1) * KT],
                            qT[h * DH:(h + 1) * DH, j * QC:(j + 1) * QC],
                            start=True, stop=True,
                        )
                    return sc

                def emit_exp(m, sc0, sc1):
                    if load["act"] <= load["dve"]:
                        # exact exp -> fp8, consumed by a DoubleRow PV
                        ex = expf8.tile([128, 2, HPC, QC], F8, tag="ex8")
                        for i, sc in enumerate((sc0, sc1)):
                            nc.scalar.activation(
                                ex[:, i, :, :], sc[:],
                                mybir.ActivationFunctionType.Exp, scale=SCALE)
                        load["act"] += ACT_MACRO_NS
                        ex_of[m] = ("dr", ex)
                    else:
                        # Schraudolph bit-trick exp -> bf16, normal PVs
                        exs = []
                        for i, sc in enumerate((sc0, sc1)):
                            ex = expp.tile([128, HPC, QC], BF16, tag="ex",
                                           name=f"ex{m}_{i}")
                            nc.vector.tensor_scalar(
                                ex[:].bitcast(I16), sc[:], EXPA, EXPB,
                                mybir.AluOpType.mult, mybir.AluOpType.add)
                            exs.append(ex)
                        load["dve"] += DVE_MACRO_NS
                        ex_of[m] = ("nrm", exs)

                def emit_pv(m):
                    j, t0 = slots[2 * m]
                    if j not in pv:
                        pv[j] = psV.tile([128, HPC, QC], F32, tag="pv",
                                         name=f"pv{j}")
                    kind, ex = ex_of.pop(m)
                    if kind == "dr":
                        for h in range(HPC):
                            nc.tensor.matmul(
                                pv[j][0:DH + 1, h, :],
                                vt[:, t0:t0 + 2, h * VHB:h * VHB + DH + 1],
                                ex[:, :, h, :],
                                start=(t0 == 0), stop=(t0 + 1 == NKT - 1),
                                perf_mode=mybir.MatmulPerfMode.DoubleRow,
                            )
                    else:
                        for i in range(2):
                            for h in range(HPC):
                                nc.tensor.matmul(
                                    pv[j][0:DH + 1, h, :],
                                    vt[:, t0 + i, h * VHB:h * VHB + DH + 1],
                                    ex[i][:, h, :],
                                    start=(t0 + i == 0),
                                    stop=(t0 + i == NKT - 1),
                                )
                    if t0 + 1 == NKT - 1:
                        emit_epilogue(j)

                def emit_epilogue(j):
                    if j == NQ - 1:
                        # latency-optimized per-head pipeline for the last j
                        den = [invL.tile([1, QC], F32, tag=f"denL{h}",
                                         name=f"denL{h}") for h in range(HPC)]
                        inv = [invL.tile([1, QC], F32, tag=f"invL{h}",
                                         name=f"invL{h}") for h in range(HPC)]
                        invb = [invL.tile([DH, QC], F32, tag=f"invbL{h}",
                                          name=f"invbL{h}") for h in range(HPC)]
                        an = [invL.tile([DH, QC], BF16, tag=f"anL{h}",
                                        name=f"anL{h}") for h in range(HPC)]
                        nc.vector.tensor_copy(den[0][:], pv[j][DH:DH + 1, 0, :])
                        nc.vector.reciprocal_approx_fast(inv[0][:], den[0][:])
                        nc.gpsimd.partition_broadcast(invb[0][:], inv[0][:])
                        nc.vector.tensor_copy(den[1][:], pv[j][DH:DH + 1, 1, :])
                        nc.vector.reciprocal_approx_fast(inv[1][:], den[1][:])
                        nc.vector.tensor_mul(an[0][:], pv[j][0:DH, 0, :],
                                             invb[0][:])
                        nc.gpsimd.partition_broadcast(invb[1][:], inv[1][:])
                        nc.sync.dma_start(a2a_in[j, 0:DH, :], an[0][:])
                        nc.vector.tensor_mul(an[1][:], pv[j][0:DH, 1, :],
                                             invb[1][:])
                        nc.sync.dma_start(a2a_in[j, DH:ICB, :], an[1][:])
                    else:
                        den = invp.tile([1, HPC, QC], F32, tag="den",
                                        name=f"den{j}")
                        inv = invp.tile([1, HPC, QC], F32, tag="inv",
                                        name=f"inv{j}")
                        invb = invp.tile([DH, HPC, QC], F32, tag="invb",
                                         name=f"invb{j}")
                        an = attp.tile([DH, HPC, QC], BF16, tag="an",
                                       name=f"an{j}")
                        nc.vector.tensor_copy(den[:], pv[j][DH:DH + 1, :, :])
                        nc.vector.reciprocal_approx_fast(inv[:], den[:])
                        for h in range(HPC):
                            nc.gpsimd.partition_broadcast(invb[:, h, :],
                                                          inv[:, h, :])
                        nc.vector.tensor_mul(an[:], pv[j][0:DH, :, :], invb[:])
                        load["dve"] += EPI_DVE_NS
                        for h in range(HPC):
                            nc.sync.dma_start(a2a_in[j, h * DH:(h + 1) * DH, :],
                                              an[:, h, :])
                    del pv[j]
                    if j == NQ - 2:
                        # re-sync cores while the last q-chunk computes
                        bar2_i = dramp.tile([1, 16], F32, tag="bar2_i")
                        bar2_o = dramp.tile([1, 16], F32, tag="bar2_o",
                                            addr_space="Shared")
                        nc.gpsimd.dma_start(bar2_i[:], a2a_in[j, 0:1, 0:16])
                        nc.gpsimd.collective_compute(
                            "AllReduce", mybir.AluOpType.add,
                            replica_groups=[list(range(NCORES))],
                            ins=[bar2_i.opt()], outs=[bar2_o.opt()],
                        )

                # macro loop: 2 score-groups, 2 exps, then a lagged PV macro
                for m in range(NM):
                    sc0 = emit_scores(2 * m)
                    sc1 = emit_scores(2 * m + 1)
                    emit_exp(m, sc0, sc1)
                    if m - LAGM >= 0:
                        emit_pv(m - LAGM)
                for m in range(NM - LAGM, NM):
                    emit_pv(m)

            # ---- exchange: my (2 heads x all seq) -> (all inner x my seq) ----
            nc.gpsimd.collective_compute(
                "AllToAll", mybir.AluOpType.bypass,
                replica_groups=[list(range(NCORES))],
                ins=[a2a_in.opt()], outs=[a2a_out.opt()],
            )

            # ---- output projection for my SEQC rows ----
            with (
                tc.tile_pool(name="psC", bufs=2, space="PSUM") as psC,
                tc.tile_pool(name="psW", bufs=1, space="PSUM") as psW,
                tc.tile_pool(name="afp", bufs=1) as afp,
                tc.tile_pool(name="finp", bufs=2) as finp,
            ):
                # keep the PE warm through the AllToAll wait
                w2_ps = psW.tile([128, QC], F32, tag="warm2")
                for _ in range(150):
                    nc.tensor.matmul(w2_ps[:], wz[:, 0:128], wz[:],
                                     start=True, stop=True)
                af = afp.tile([128, NCORES, QC], BF16, tag="af")
                for r in range(NCORES):
                    (nc.sync if r % 2 == 0 else nc.scalar).dma_start(
                        af[:, r, :], a2a_out[r])
                bo3 = bo_t[:].rearrange("p (a b) -> p a b", a=2)
                for s in range(SEQC // 128):
                    yps = psC.tile([128, 2, QC], F32, tag="y")
                    for r in range(NCORES):
                        for half in range(2):
                            nc.tensor.matmul(
                                yps[:, half, :],
                                af[:, r, s * 128:(s + 1) * 128],
                                wo_t[:, r, half * QC:(half + 1) * QC],
                                start=(r == 0), stop=(r == NCORES - 1))
                    ysb = finp.tile([128, 2, QC], F32, tag="ysb")
                    nc.vector.tensor_add(ysb[:], yps[:], bo3)
                    orows = out[s * 128:(s + 1) * 128, :].rearrange(
                        "p (a b) -> p a b", a=2)
                    for half in range(2):
                        (nc.sync if half == 0 else nc.scalar).dma_start(
                            orows[:, half, :], ysb[:, half, :])

    nc.compile()
    return nc


_NC_CACHE = None


def _get_nc():
    global _NC_CACHE
    if _NC_CACHE is None:
        _NC_CACHE = build_kernel()
    return _NC_CACHE


def _prep_inputs(x, Wq, Wk, Wv, Wo, bo):
    """Host-side sharding/layout prep (untimed)."""
    # [128 dim-rows, piece, dim-chunk, 512 seq-cols]
    xt_p = np.ascontiguousarray(
        x.T.reshape(DCH, 128, NQ, QC).transpose(1, 2, 0, 3)).astype(BF16_NP)
    wo_p = np.ascontiguousarray(
        Wo.reshape(DCH, 128, DIM).transpose(1, 0, 2)).astype(BF16_NP)
    bo_p = np.ascontiguousarray(np.tile(bo[None, :], (128, 1))).astype(np.float32)
    in_maps = []
    for c in range(NCORES):
        ic = slice(c * ICB, (c + 1) * ICB)
        m = {"xt": xt_p, "wo": wo_p, "bo": bo_p}
        for name, W in (("wq", Wq), ("wk", Wk), ("wv", Wv)):
            m[name] = np.ascontiguousarray(
                W[:, ic].reshape(DCH, 128, ICB).transpose(1, 0, 2)).astype(BF16_NP)
        in_maps.append(m)
    return in_maps


def kernel(x, Wq, Wk, Wv, Wo, bo, _trace=False):
    x = np.asarray(x, np.float32)
    Wq = np.asarray(Wq, np.float32)
    Wk = np.asarray(Wk, np.float32)
    Wv = np.asarray(Wv, np.float32)
    Wo = np.asarray(Wo, np.float32)
    bo = np.asarray(bo, np.float32)
    nc = _get_nc()
    in_maps = _prep_inputs(x, Wq, Wk, Wv, Wo, bo)
    r = run_bass_kernel_spmd(nc, in_maps, core_ids=list(range(NCORES)),
                             trace=_trace)
    y = np.concatenate([r.results[c]["out"] for c in range(NCORES)], axis=0)
    if _trace:
        kernel.last_result = r
    return y.astype(np.float32)


# revision 27
# speedup vs baseline: 1.0518x; 1.0373x over previous
"""Distributed multi-head attention for TRN2, 8 NeuronCores.

Sharding: tensor-parallel over heads (2 heads / core) for QKV + attention;
an AllToAll exchanges normalized attention outputs so each core computes
the output projection for its own 512 sequence rows.

Perf structure (v3):
- input x streamed piece-major (8 seq-pieces x all dim-chunks, 4KB/
  partition lines) on BOTH hardware DGE queues (sync + scalar).
- scores emitted as (h0, h1) pairs -> disjoint PE quadrants run them
  concurrently; macros of 2 key-tiles keep same-shape matmul chains
  long so LDWEIGHTS stays hidden.
- PV matmuls lag scores by LAG groups: exp latency (either engine)
  never gates the PE.
- exp split across ScalarE (exact LUT) and VectorE (Schraudolph
  bit-trick: i16 = round(s*A + B) reinterpreted as bf16) with
  credit-based balancing.
- dummy matmuls keep the PE clock warm across the AllToAll wait.
"""
import numpy as np
import ml_dtypes

import concourse.bass as bass
import concourse.tile as tile
from concourse import bacc, mybir
from concourse.bass_utils import run_bass_kernel_spmd

# problem dims (hardcoded; kernel.py must be self-contained)
N, DIM, HEADS, DH = 4096, 1024, 16, 64
NCORES = 8
HPC = HEADS // NCORES        # 2 heads per core
ICB = HPC * DH               # 128 inner dims per core
DCH = DIM // 128             # 8 dim chunks
QC = 512                     # query-chunk (columns per scores matmul)
NQ = N // QC                 # 8
KT = 128                     # key tile (scores output partitions)
NKT = N // KT                # 32
SEQC = N // NCORES           # 512 output rows per core
SCALE = float(DH) ** -0.5
LAG = 8                      # PV pipeline lag (groups)

# Schraudolph-exp constants for the DVE share: bf16 bits of exp(s*SCALE)
# ~= round(s * EXPA + EXPB); worst-case rel err ~3.3%, which the softmax
# normalization mostly cancels (measured end-to-end ~7e-3 absmax-rel).
EXPA = SCALE * 1.4426950408889634 * 128.0
EXPB = 16256.0 - 5.35

BF16 = mybir.dt.bfloat16
F32 = mybir.dt.float32
F8 = mybir.dt.float8e4
I16 = mybir.dt.int16
BF16_NP = ml_dtypes.bfloat16

# v layout: per-head block padded to 80 cols (65 used) so DoubleRow
# access-pattern strides are 16-aligned; t-stride 160
VHB = 80
# per-MACRO exp cost (ns) for ACT/DVE load balancing at trace time.
# macro = 2 key-tile groups: ACT macros get exact exp + fp8 DoubleRow PV,
# DVE macros get Schraudolph exp + bf16-rhs PV.
ACT_MACRO_NS = 2 * (1024 + 352) / 1.2
DVE_MACRO_NS = 2 * (1024 + 271) / 0.96
EPI_DVE_NS = 3400.0          # den copy + recip + an-mul per j on DVE


def build_kernel():
    nc = bacc.Bacc("TRN2", target_bir_lowering=False, debug=False,
                   enable_asserts=True, num_devices=NCORES)

    # x piece-major: [128 dim-rows, piece, dim-chunk, 512 seq]
    xt = nc.dram_tensor("xt", [128, NQ, DCH, QC], BF16, kind="ExternalInput")
    wq = nc.dram_tensor("wq", [128, DCH, ICB], BF16, kind="ExternalInput")
    wk = nc.dram_tensor("wk", [128, DCH, ICB], BF16, kind="ExternalInput")
    wv = nc.dram_tensor("wv", [128, DCH, ICB], BF16, kind="ExternalInput")
    wo = nc.dram_tensor("wo", [128, DCH, DIM], BF16, kind="ExternalInput")
    bo = nc.dram_tensor("bo", [128, DIM], F32, kind="ExternalInput")
    out = nc.dram_tensor("out", [SEQC, DIM], F32, kind="ExternalOutput")
    wsink = nc.dram_tensor("warm_sink", [128, 16], F32, kind="ExternalOutput")

    with tile.TileContext(nc) as tc:
        with (
            tc.tile_pool(name="xtp", bufs=NQ) as xtp,
            tc.tile_pool(name="wp", bufs=1) as wp,
            tc.tile_pool(name="qk", bufs=1) as qkp,
            tc.tile_pool(name="dram", bufs=1, space="DRAM") as dramp,
        ):
            # ---- load inputs: weights first (k-proj gate), then x pieces
            # split across BOTH hardware DGE queues ----
            wq_t = wp.tile([128, DCH, ICB], BF16, tag="wq")
            wk_t = wp.tile([128, DCH, ICB], BF16, tag="wk")
            wv_t = wp.tile([128, DCH, ICB], BF16, tag="wv")
            wo_t = wp.tile([128, DCH, DIM], BF16, tag="wo")
            bo_t = wp.tile([128, DIM], F32, tag="bo")
            nc.sync.dma_start(wk_t[:], wk[:])
            nc.scalar.dma_start(wq_t[:], wq[:])
            nc.scalar.dma_start(wv_t[:], wv[:])
            xt_t = [xtp.tile([128, DCH, QC], BF16, tag="xt", name=f"xt{p}")
                    for p in range(NQ)]
            for p in range(NQ):
                half = DCH // 2
                nc.sync.dma_start(xt_t[p][:, 0:half, :], xt[:, p, 0:half, :])
                nc.scalar.dma_start(xt_t[p][:, half:DCH, :],
                                    xt[:, p, half:DCH, :])
            # wo/bo on the sync queue only: the scalar engine's DMA issues
            # serialize with transfer completions, and anything queued there
            # would delay the first exps (in-order engine queue)
            for d in range(DCH):
                nc.sync.dma_start(wo_t[:, d, :], wo[:, d, :])
            nc.sync.dma_start(bo_t[:], bo[:])

            # early barrier: absorb inter-core startup skew during the ramp
            bar_i = dramp.tile([1, 16], F32, tag="bar_i")
            bar_o = dramp.tile([1, 16], F32, tag="bar_o", addr_space="Shared")
            nc.gpsimd.dma_start(bar_i[:], wk[0:1, 0, 0:16])
            nc.gpsimd.collective_compute(
                "AllReduce", mybir.AluOpType.add,
                replica_groups=[list(range(NCORES))],
                ins=[bar_i.opt()], outs=[bar_o.opt()],
            )

            qT = qkp.tile([128, N], BF16, tag="qT")   # [2 heads x 64, seq]
            kT = qkp.tile([128, N], BF16, tag="kT")
            # v natural layout + ones column per head, fp8 (PV runs fp8;
            # exact for the DoubleRow share, bf16-rhs for the rest)
            vt = qkp.tile([128, NKT, HPC * VHB], F8, tag="vt")
            nc.gpsimd.memset(vt[:], 1.0)

            a2a_in = dramp.tile([NCORES, ICB, QC], BF16, tag="a2a_in")
            a2a_out = dramp.tile([NCORES, ICB, QC], BF16, tag="a2a_out")

            wz = wp.tile([128, QC], BF16, tag="wz")
            nc.gpsimd.memset(wz[:], 0.0)

            # ---- projections per piece: k block, q block, v tiles ----
            with tc.tile_pool(name="psA", bufs=4, space="PSUM") as psA:
                # warm-up: dep-free matmuls while the first DMAs stream
                w_ps = psA.tile([128, QC], F32, tag="warm", name="warm_ps")
                last_warm = None
                for _ in range(32):
                    last_warm = nc.tensor.matmul(w_ps[:], wz[:, 0:128], wz[:],
                                                 start=True, stop=True)
                wcp = wp.tile([128, 16], F32, tag="wcp")
                nc.vector.tensor_copy(wcp[:], w_ps[:, 0:16])
                nc.sync.dma_start(wsink[:], wcp[:])

                first_real = None
                for p in range(NQ):
                    for dst, w_t in ((kT, wk_t), (qT, wq_t)):
                        ps = psA.tile([128, QC], F32, tag="proj",
                                      name=f"pj{p}_{0 if dst is kT else 1}")
                        for d in range(DCH):
                            m = nc.tensor.matmul(
                                ps[:], w_t[:, d, :], xt_t[p][:, d, :],
                                start=(d == 0), stop=(d == DCH - 1))
                            if first_real is None:
                                first_real = m
                        # DVE copy: ScalarE's queue is busy issuing piece DMAs
                        # (in-order engine; a copy there stalls the proj ring)
                        nc.vector.tensor_copy(dst[:, p * QC:(p + 1) * QC], ps[:])
                    # v tiles for this piece (natural layout)
                    vs = psA.tile([128, QC], F32, tag="proj", name=f"pv{p}")
                    vs4 = vs[:].rearrange("p (t k) -> p t k", t=4)
                    for tt in range(4):
                        for d in range(DCH):
                            nc.tensor.matmul(
                                vs4[:, tt, :],
                                xt_t[p][:, d, tt * KT:(tt + 1) * KT],
                                wv_t[:, d, :],
                                start=(d == 0), stop=(d == DCH - 1))
                    for tt in range(4):
                        t = 4 * p + tt
                        nc.vector.tensor_copy(vt[:, t, 0:DH], vs4[:, tt, 0:DH])
                        nc.vector.tensor_copy(vt[:, t, VHB:VHB + DH],
                                              vs4[:, tt, DH:ICB])
                bass._add_dep_helper(first_real.ins, last_warm.ins, sync=False,
                                     reason="warm-up runs before projections")

            with (
                tc.tile_pool(name="psS", bufs=2, space="PSUM") as psS,
                tc.tile_pool(name="psV", bufs=2, space="PSUM") as psV,
                tc.tile_pool(name="expp", bufs=14) as expp,
                tc.tile_pool(name="expf8", bufs=7) as expf8,
                tc.tile_pool(name="attp", bufs=2) as attp,
                tc.tile_pool(name="invp", bufs=2) as invp,
                tc.tile_pool(name="invL", bufs=1) as invL,
            ):
                # ---- attention: macros of 2 key-tiles; PV lags LAGM macros
                # behind scores/exp so exp latency never gates the PE ----
                slots = [(j, t) for j in range(NQ) for t in range(NKT)]
                NM = len(slots) // 2        # macros; never straddle j
                LAGM = LAG // 2
                pv = {}
                ex_of = {}
                load = {"act": 0.0, "dve": 0.0}

                def emit_scores(s):
                    j, t = slots[s]
                    sc = psS.tile([128, HPC, QC], F32, tag="sc")
                    for h in range(HPC):
                        nc.tensor.matmul(
                            sc[:, h, :],
                            kT[h * DH:(h + 1) * DH, t * KT:(t + 1) * KT],
                            qT[h * DH:(h + 1) * DH, j * QC:(j + 1) * QC],
                            start=True, stop=True,
                        )
                    return sc

                def emit_exp(m, sc0, sc1):
                    # last macros forced to ACT: shortest latency into the
                    # j=7 epilogue -> AllToAll chain
                    if m >= NM - 2 or load["act"] <= load["dve"]:
                        # exact exp -> fp8, consumed by a DoubleRow PV
                        ex = expf8.tile([128, 2, HPC, QC], F8, tag="ex8")
                        for i, sc in enumerate((sc0, sc1)):
                            nc.scalar.activation(
                                ex[:, i, :, :], sc[:],
                                mybir.ActivationFunctionType.Exp, scale=SCALE)
                        load["act"] += ACT_MACRO_NS
                        ex_of[m] = ("dr", ex)
                    else:
                        # Schraudolph bit-trick exp -> bf16, normal PVs
                        exs = []
                        for i, sc in enumerate((sc0, sc1)):
                            ex = expp.tile([128, HPC, QC], BF16, tag="ex",
                                           name=f"ex{m}_{i}")
                            nc.vector.tensor_scalar(
                                ex[:].bitcast(I16), sc[:], EXPA, EXPB,
                                mybir.AluOpType.mult, mybir.AluOpType.add)
                            exs.append(ex)
                        load["dve"] += DVE_MACRO_NS
                        ex_of[m] = ("nrm", exs)

                def emit_pv(m):
                    j, t0 = slots[2 * m]
                    if j not in pv:
                        pv[j] = psV.tile([128, HPC, QC], F32, tag="pv",
                                         name=f"pv{j}")
                    kind, ex = ex_of.pop(m)
                    if kind == "dr":
                        for h in range(HPC):
                            nc.tensor.matmul(
                                pv[j][0:DH + 1, h, :],
                                vt[:, t0:t0 + 2, h * VHB:h * VHB + DH + 1],
                                ex[:, :, h, :],
                                start=(t0 == 0), stop=(t0 + 1 == NKT - 1),
                                perf_mode=mybir.MatmulPerfMode.DoubleRow,
                            )
                    else:
                        for i in range(2):
                            for h in range(HPC):
                                nc.tensor.matmul(
                                    pv[j][0:DH + 1, h, :],
                                    vt[:, t0 + i, h * VHB:h * VHB + DH + 1],
                                    ex[i][:, h, :],
                                    start=(t0 + i == 0),
                                    stop=(t0 + i == NKT - 1),
                                )
                    if t0 + 1 == NKT - 1:
                        emit_epilogue(j)

                def emit_epilogue(j):
                    if j == NQ - 1:
                        # latency-optimized per-head pipeline for the last j
                        den = [invL.tile([1, QC], F32, tag=f"denL{h}",
                                         name=f"denL{h}") for h in range(HPC)]
                        inv = [invL.tile([1, QC], F32, tag=f"invL{h}",
                                         name=f"invL{h}") for h in range(HPC)]
                        invb = [invL.tile([DH, QC], F32, tag=f"invbL{h}",
                                          name=f"invbL{h}") for h in range(HPC)]
                        an = [invL.tile([DH, QC], BF16, tag=f"anL{h}",
                                        name=f"anL{h}") for h in range(HPC)]
                        nc.vector.tensor_copy(den[0][:], pv[j][DH:DH + 1, 0, :])
                        nc.vector.reciprocal_approx_fast(inv[0][:], den[0][:])
                        nc.gpsimd.partition_broadcast(invb[0][:], inv[0][:])
                        nc.vector.tensor_copy(den[1][:], pv[j][DH:DH + 1, 1, :])
                        nc.vector.reciprocal_approx_fast(inv[1][:], den[1][:])
                        nc.vector.tensor_mul(an[0][:], pv[j][0:DH, 0, :],
                                             invb[0][:])
                        nc.gpsimd.partition_broadcast(invb[1][:], inv[1][:])
                        nc.sync.dma_start(a2a_in[j, 0:DH, :], an[0][:])
                        nc.vector.tensor_mul(an[1][:], pv[j][0:DH, 1, :],
                                             invb[1][:])
                        nc.sync.dma_start(a2a_in[j, DH:ICB, :], an[1][:])
                    else:
                        den = invp.tile([1, HPC, QC], F32, tag="den",
                                        name=f"den{j}")
                        inv = invp.tile([1, HPC, QC], F32, tag="inv",
                                        name=f"inv{j}")
                        invb = invp.tile([DH, HPC, QC], F32, tag="invb",
                                         name=f"invb{j}")
                        an = attp.tile([DH, HPC, QC], BF16, tag="an",
                                       name=f"an{j}")
                        nc.vector.tensor_copy(den[:], pv[j][DH:DH + 1, :, :])
                        nc.vector.reciprocal_approx_fast(inv[:], den[:])
                        for h in range(HPC):
                            nc.gpsimd.partition_broadcast(invb[:, h, :],
                                                          inv[:, h, :])
                        nc.vector.tensor_mul(an[:], pv[j][0:DH, :, :], invb[:])
                        load["dve"] += EPI_DVE_NS
                        for h in range(HPC):
                            nc.sync.dma_start(a2a_in[j, h * DH:(h + 1) * DH, :],
                                              an[:, h, :])
                    del pv[j]
                    if j == NQ - 2:
                        # re-sync cores while the last q-chunk computes
                        bar2_i = dramp.tile([1, 16], F32, tag="bar2_i")
                        bar2_o = dramp.tile([1, 16], F32, tag="bar2_o",
                                            addr_space="Shared")
                        nc.gpsimd.dma_start(bar2_i[:], a2a_in[j, 0:1, 0:16])
                        nc.gpsimd.collective_compute(
                            "AllReduce", mybir.AluOpType.add,
                            replica_groups=[list(range(NCORES))],
                            ins=[bar2_i.opt()], outs=[bar2_o.opt()],
                        )

                # macro loop: 2 score-groups, 2 exps, then a lagged PV macro
                for m in range(NM):
                    sc0 = emit_scores(2 * m)
                    sc1 = emit_scores(2 * m + 1)
                    emit_exp(m, sc0, sc1)
                    if m - LAGM >= 0:
                        emit_pv(m - LAGM)
                for m in range(NM - LAGM, NM):
                    emit_pv(m)

                # keep the PE warm through the pool transition + AllToAll
                # wait: dep-free dummies INSIDE this pool scope start the
                # instant the last PV drains (no pool-close DRAIN gap), so
                # HAM never re-throttles before the output projection
                sc_w = psS.tile([128, HPC, QC], F32, tag="sc", name="sc_warm")
                for _ in range(150):
                    nc.tensor.matmul(sc_w[:, 0, :], wz[:, 0:128], wz[:],
                                     start=True, stop=True)

            # ---- exchange: my (2 heads x all seq) -> (all inner x my seq) ----
            nc.gpsimd.collective_compute(
                "AllToAll", mybir.AluOpType.bypass,
                replica_groups=[list(range(NCORES))],
                ins=[a2a_in.opt()], outs=[a2a_out.opt()],
            )

            # ---- output projection for my SEQC rows ----
            with (
                tc.tile_pool(name="psC", bufs=2, space="PSUM") as psC,
                tc.tile_pool(name="afp", bufs=1) as afp,
                tc.tile_pool(name="finp", bufs=2) as finp,
            ):
                af = afp.tile([128, NCORES, QC], BF16, tag="af")
                for r in range(NCORES):
                    (nc.sync if r % 2 == 0 else nc.scalar).dma_start(
                        af[:, r, :], a2a_out[r])
                bo3 = bo_t[:].rearrange("p (a b) -> p a b", a=2)
                for s in range(SEQC // 128):
                    yps = psC.tile([128, 2, QC], F32, tag="y")
                    for r in range(NCORES):
                        for half in range(2):
                            nc.tensor.matmul(
                                yps[:, half, :],
                                af[:, r, s * 128:(s + 1) * 128],
                                wo_t[:, r, half * QC:(half + 1) * QC],
                                start=(r == 0), stop=(r == NCORES - 1))
                    ysb = finp.tile([128, 2, QC], F32, tag="ysb")
                    nc.vector.tensor_add(ysb[:], yps[:], bo3)
                    orows = out[s * 128:(s + 1) * 128, :].rearrange(
                        "p (a b) -> p a b", a=2)
                    for half in range(2):
                        (nc.sync if half == 0 else nc.scalar).dma_start(
                            orows[:, half, :], ysb[:, half, :])

    nc.compile()
    return nc


_NC_CACHE = None


def _get_nc():
    global _NC_CACHE
    if _NC_CACHE is None:
        _NC_CACHE = build_kernel()
    return _NC_CACHE


def _prep_inputs(x, Wq, Wk, Wv, Wo, bo):
    """Host-side sharding/layout prep (untimed)."""
    # [128 dim-rows, piece, dim-chunk, 512 seq-cols]
    xt_p = np.ascontiguousarray(
        x.T.reshape(DCH, 128, NQ, QC).transpose(1, 2, 0, 3)).astype(BF16_NP)
    wo_p = np.ascontiguousarray(
        Wo.reshape(DCH, 128, DIM).transpose(1, 0, 2)).astype(BF16_NP)
    bo_p = np.ascontiguousarray(np.tile(bo[None, :], (128, 1))).astype(np.float32)
    in_maps = []
    for c in range(NCORES):
        ic = slice(c * ICB, (c + 1) * ICB)
        m = {"xt": xt_p, "wo": wo_p, "bo": bo_p}
        for name, W in (("wq", Wq), ("wk", Wk), ("wv", Wv)):
            m[name] = np.ascontiguousarray(
                W[:, ic].reshape(DCH, 128, ICB).transpose(1, 0, 2)).astype(BF16_NP)
        in_maps.append(m)
    return in_maps


def kernel(x, Wq, Wk, Wv, Wo, bo, _trace=False):
    x = np.asarray(x, np.float32)
    Wq = np.asarray(Wq, np.float32)
    Wk = np.asarray(Wk, np.float32)
    Wv = np.asarray(Wv, np.float32)
    Wo = np.asarray(Wo, np.float32)
    bo = np.asarray(bo, np.float32)
    nc = _get_nc()
    in_maps = _prep_inputs(x, Wq, Wk, Wv, Wo, bo)
    r = run_bass_kernel_spmd(nc, in_maps, core_ids=list(range(NCORES)),
                             trace=_trace)
    y = np.concatenate([r.results[c]["out"] for c in range(NCORES)], axis=0)
    if _trace:
        kernel.last_result = r
    return y.astype(np.float32)


# revision 29
# speedup vs baseline: 1.0696x; 1.0170x over previous
"""Distributed multi-head attention for TRN2, 8 NeuronCores.

Sharding: tensor-parallel over heads (2 heads / core) for QKV + attention;
an AllToAll exchanges normalized attention outputs so each core computes
the output projection for its own 512 sequence rows.

Perf structure (v3):
- input x streamed piece-major (8 seq-pieces x all dim-chunks, 4KB/
  partition lines) on BOTH hardware DGE queues (sync + scalar).
- scores emitted as (h0, h1) pairs -> disjoint PE quadrants run them
  concurrently; macros of 2 key-tiles keep same-shape matmul chains
  long so LDWEIGHTS stays hidden.
- PV matmuls lag scores by LAG groups: exp latency (either engine)
  never gates the PE.
- exp split across ScalarE (exact LUT) and VectorE (Schraudolph
  bit-trick: i16 = round(s*A + B) reinterpreted as bf16) with
  credit-based balancing.
- dummy matmuls keep the PE clock warm across the AllToAll wait.
"""
import numpy as np
import ml_dtypes

import concourse.bass as bass
import concourse.tile as tile
from concourse import bacc, mybir
from concourse.bass_utils import run_bass_kernel_spmd

# problem dims (hardcoded; kernel.py must be self-contained)
N, DIM, HEADS, DH = 4096, 1024, 16, 64
NCORES = 8
HPC = HEADS // NCORES        # 2 heads per core
ICB = HPC * DH               # 128 inner dims per core
DCH = DIM // 128             # 8 dim chunks
QC = 512                     # query-chunk (columns per scores matmul)
NQ = N // QC                 # 8
KT = 128                     # key tile (scores output partitions)
NKT = N // KT                # 32
SEQC = N // NCORES           # 512 output rows per core
SCALE = float(DH) ** -0.5
LAG = 8                      # PV pipeline lag (groups)

# Schraudolph-exp constants for the DVE share: bf16 bits of exp(s*SCALE)
# ~= round(s * EXPA + EXPB); worst-case rel err ~3.3%, which the softmax
# normalization mostly cancels (measured end-to-end ~7e-3 absmax-rel).
EXPA = SCALE * 1.4426950408889634 * 128.0
EXPB = 16256.0 - 5.35

BF16 = mybir.dt.bfloat16
F32 = mybir.dt.float32
F8 = mybir.dt.float8e4
I16 = mybir.dt.int16
BF16_NP = ml_dtypes.bfloat16

# v layout: per-head block padded to 80 cols (65 used) so DoubleRow
# access-pattern strides are 16-aligned; t-stride 160
VHB = 80
# per-MACRO exp cost (ns) for ACT/DVE load balancing at trace time.
# macro = 2 key-tile groups: ACT macros get exact exp + fp8 DoubleRow PV,
# DVE macros get Schraudolph exp + bf16-rhs PV.
ACT_MACRO_NS = 2 * (1024 + 352) / 1.2
DVE_MACRO_NS = 2 * (1024 + 271) / 0.96
EPI_DVE_NS = 3400.0          # den copy + recip + an-mul per j on DVE


def build_kernel():
    nc = bacc.Bacc("TRN2", target_bir_lowering=False, debug=False,
                   enable_asserts=True, num_devices=NCORES)

    # x piece-major: [128 dim-rows, piece, dim-chunk, 512 seq]
    xt = nc.dram_tensor("xt", [128, NQ, DCH, QC], BF16, kind="ExternalInput")
    wq = nc.dram_tensor("wq", [128, DCH, ICB], BF16, kind="ExternalInput")
    wk = nc.dram_tensor("wk", [128, DCH, ICB], BF16, kind="ExternalInput")
    wv = nc.dram_tensor("wv", [128, DCH, ICB], BF16, kind="ExternalInput")
    wo = nc.dram_tensor("wo", [128, DCH, DIM], BF16, kind="ExternalInput")
    bo = nc.dram_tensor("bo", [128, DIM], F32, kind="ExternalInput")
    out = nc.dram_tensor("out", [SEQC, DIM], F32, kind="ExternalOutput")
    wsink = nc.dram_tensor("warm_sink", [128, 16], F32, kind="ExternalOutput")

    with tile.TileContext(nc) as tc:
        with (
            tc.tile_pool(name="xtp", bufs=NQ) as xtp,
            tc.tile_pool(name="wp", bufs=1) as wp,
            tc.tile_pool(name="qk", bufs=1) as qkp,
            tc.tile_pool(name="dram", bufs=1, space="DRAM") as dramp,
        ):
            # ---- load inputs: weights first (k-proj gate), then x pieces
            # split across BOTH hardware DGE queues ----
            wq_t = wp.tile([128, DCH, ICB], BF16, tag="wq")
            wk_t = wp.tile([128, DCH, ICB], BF16, tag="wk")
            wv_t = wp.tile([128, DCH, ICB], BF16, tag="wv")
            wo_t = wp.tile([128, DCH, DIM], BF16, tag="wo")
            bo_t = wp.tile([128, DIM], F32, tag="bo")
            nc.sync.dma_start(wk_t[:], wk[:])
            nc.scalar.dma_start(wq_t[:], wq[:])
            nc.scalar.dma_start(wv_t[:], wv[:])
            xt_t = [xtp.tile([128, DCH, QC], BF16, tag="xt", name=f"xt{p}")
                    for p in range(NQ)]
            for p in range(NQ):
                half = DCH // 2
                nc.sync.dma_start(xt_t[p][:, 0:half, :], xt[:, p, 0:half, :])
                nc.scalar.dma_start(xt_t[p][:, half:DCH, :],
                                    xt[:, p, half:DCH, :])
            # wo/bo on the sync queue only: the scalar engine's DMA issues
            # serialize with transfer completions, and anything queued there
            # would delay the first exps (in-order engine queue)
            for d in range(DCH):
                nc.sync.dma_start(wo_t[:, d, :], wo[:, d, :])
            nc.sync.dma_start(bo_t[:], bo[:])

            # early barrier: absorb inter-core startup skew during the ramp
            bar_i = dramp.tile([1, 16], F32, tag="bar_i")
            bar_o = dramp.tile([1, 16], F32, tag="bar_o", addr_space="Shared")
            nc.gpsimd.dma_start(bar_i[:], wk[0:1, 0, 0:16])
            nc.gpsimd.collective_compute(
                "AllReduce", mybir.AluOpType.add,
                replica_groups=[list(range(NCORES))],
                ins=[bar_i.opt()], outs=[bar_o.opt()],
            )

            qT = qkp.tile([128, N], BF16, tag="qT")   # [2 heads x 64, seq]
            kT = qkp.tile([128, N], BF16, tag="kT")
            # v natural layout + ones column per head, fp8 (PV runs fp8;
            # exact for the DoubleRow share, bf16-rhs for the rest)
            vt = qkp.tile([128, NKT, HPC * VHB], F8, tag="vt")
            nc.gpsimd.memset(vt[:], 1.0)

            a2a_in = dramp.tile([NCORES, ICB, QC], BF16, tag="a2a_in")
            a2a_out = dramp.tile([NCORES, ICB, QC], BF16, tag="a2a_out")

            wz = wp.tile([128, QC], BF16, tag="wz")
            nc.gpsimd.memset(wz[:], 0.0)

            # ---- projections per piece: k block, q block, v tiles ----
            with tc.tile_pool(name="psA", bufs=4, space="PSUM") as psA:
                # warm-up: dep-free matmuls while the first DMAs stream
                w_ps = psA.tile([128, QC], F32, tag="warm", name="warm_ps")
                last_warm = None
                for _ in range(32):
                    last_warm = nc.tensor.matmul(w_ps[:], wz[:, 0:128], wz[:],
                                                 start=True, stop=True)
                wcp = wp.tile([128, 16], F32, tag="wcp")
                nc.vector.tensor_copy(wcp[:], w_ps[:, 0:16])
                nc.sync.dma_start(wsink[:], wcp[:])

                first_real = None
                for p in range(NQ):
                    for dst, w_t in ((kT, wk_t), (qT, wq_t)):
                        ps = psA.tile([128, QC], F32, tag="proj",
                                      name=f"pj{p}_{0 if dst is kT else 1}")
                        for d in range(DCH):
                            m = nc.tensor.matmul(
                                ps[:], w_t[:, d, :], xt_t[p][:, d, :],
                                start=(d == 0), stop=(d == DCH - 1))
                            if first_real is None:
                                first_real = m
                        # DVE copy: ScalarE's queue is busy issuing piece DMAs
                        # (in-order engine; a copy there stalls the proj ring)
                        nc.vector.tensor_copy(dst[:, p * QC:(p + 1) * QC], ps[:])
                    # v tiles for this piece (natural layout)
                    vs = psA.tile([128, QC], F32, tag="proj", name=f"pv{p}")
                    vs4 = vs[:].rearrange("p (t k) -> p t k", t=4)
                    for tt in range(4):
                        for d in range(DCH):
                            nc.tensor.matmul(
                                vs4[:, tt, :],
                                xt_t[p][:, d, tt * KT:(tt + 1) * KT],
                                wv_t[:, d, :],
                                start=(d == 0), stop=(d == DCH - 1))
                    for tt in range(4):
                        t = 4 * p + tt
                        nc.vector.tensor_copy(vt[:, t, 0:DH], vs4[:, tt, 0:DH])
                        nc.vector.tensor_copy(vt[:, t, VHB:VHB + DH],
                                              vs4[:, tt, DH:ICB])
                bass._add_dep_helper(first_real.ins, last_warm.ins, sync=False,
                                     reason="warm-up runs before projections")

            with (
                tc.tile_pool(name="psS", bufs=3, space="PSUM") as psS,
                tc.tile_pool(name="psV", bufs=1, space="PSUM") as psV,
                tc.tile_pool(name="expp", bufs=14) as expp,
                tc.tile_pool(name="expf8", bufs=7) as expf8,
                tc.tile_pool(name="attp", bufs=2) as attp,
                tc.tile_pool(name="invp", bufs=2) as invp,
                tc.tile_pool(name="invL", bufs=1) as invL,
            ):
                # ---- attention: macros of 2 key-tiles; PV lags LAGM macros
                # behind scores/exp so exp latency never gates the PE ----
                slots = [(j, t) for j in range(NQ) for t in range(NKT)]
                NM = len(slots) // 2        # macros; never straddle j
                LAGM = LAG // 2
                pv = {}
                ex_of = {}
                load = {"act": 0.0, "dve": 0.0}

                def emit_scores(s):
                    j, t = slots[s]
                    sc = psS.tile([128, HPC, QC], F32, tag="sc")
                    for h in range(HPC):
                        nc.tensor.matmul(
                            sc[:, h, :],
                            kT[h * DH:(h + 1) * DH, t * KT:(t + 1) * KT],
                            qT[h * DH:(h + 1) * DH, j * QC:(j + 1) * QC],
                            start=True, stop=True,
                        )
                    return sc

                def emit_exp(m, sc0, sc1):
                    # last macros forced to ACT: shortest latency into the
                    # j=7 epilogue -> AllToAll chain
                    if m >= NM - 2 or load["act"] <= load["dve"]:
                        # exact exp -> fp8, consumed by a DoubleRow PV
                        ex = expf8.tile([128, 2, HPC, QC], F8, tag="ex8")
                        for i, sc in enumerate((sc0, sc1)):
                            nc.scalar.activation(
                                ex[:, i, :, :], sc[:],
                                mybir.ActivationFunctionType.Exp, scale=SCALE)
                        load["act"] += ACT_MACRO_NS
                        ex_of[m] = ("dr", ex)
                    else:
                        # Schraudolph bit-trick exp -> bf16, normal PVs
                        exs = []
                        for i, sc in enumerate((sc0, sc1)):
                            ex = expp.tile([128, HPC, QC], BF16, tag="ex",
                                           name=f"ex{m}_{i}")
                            nc.vector.tensor_scalar(
                                ex[:].bitcast(I16), sc[:], EXPA, EXPB,
                                mybir.AluOpType.mult, mybir.AluOpType.add)
                            exs.append(ex)
                        load["dve"] += DVE_MACRO_NS
                        ex_of[m] = ("nrm", exs)

                def emit_pv(m):
                    j, t0 = slots[2 * m]
                    if j not in pv:
                        pv[j] = psV.tile([128, HPC, QC], F32, tag="pv",
                                         name=f"pv{j}")
                    kind, ex = ex_of.pop(m)
                    if kind == "dr":
                        for h in range(HPC):
                            nc.tensor.matmul(
                                pv[j][0:DH + 1, h, :],
                                vt[:, t0:t0 + 2, h * VHB:h * VHB + DH + 1],
                                ex[:, :, h, :],
                                start=(t0 == 0), stop=(t0 + 1 == NKT - 1),
                                perf_mode=mybir.MatmulPerfMode.DoubleRow,
                            )
                    else:
                        for i in range(2):
                            for h in range(HPC):
                                nc.tensor.matmul(
                                    pv[j][0:DH + 1, h, :],
                                    vt[:, t0 + i, h * VHB:h * VHB + DH + 1],
                                    ex[i][:, h, :],
                                    start=(t0 + i == 0),
                                    stop=(t0 + i == NKT - 1),
                                )
                    if t0 + 1 == NKT - 1:
                        emit_epilogue(j)

                def emit_epilogue(j):
                    if j == NQ - 1:
                        # latency-optimized per-head pipeline for the last j
                        den = [invL.tile([1, QC], F32, tag=f"denL{h}",
                                         name=f"denL{h}") for h in range(HPC)]
                        inv = [invL.tile([1, QC], F32, tag=f"invL{h}",
                                         name=f"invL{h}") for h in range(HPC)]
                        invb = [invL.tile([DH, QC], F32, tag=f"invbL{h}",
                                          name=f"invbL{h}") for h in range(HPC)]
                        an = [invL.tile([DH, QC], BF16, tag=f"anL{h}",
                                        name=f"anL{h}") for h in range(HPC)]
                        nc.vector.tensor_copy(den[0][:], pv[j][DH:DH + 1, 0, :])
                        nc.vector.reciprocal_approx_fast(inv[0][:], den[0][:])
                        nc.gpsimd.partition_broadcast(invb[0][:], inv[0][:])
                        nc.vector.tensor_copy(den[1][:], pv[j][DH:DH + 1, 1, :])
                        nc.vector.reciprocal_approx_fast(inv[1][:], den[1][:])
                        nc.vector.tensor_mul(an[0][:], pv[j][0:DH, 0, :],
                                             invb[0][:])
                        nc.gpsimd.partition_broadcast(invb[1][:], inv[1][:])
                        nc.sync.dma_start(a2a_in[j, 0:DH, :], an[0][:])
                        nc.vector.tensor_mul(an[1][:], pv[j][0:DH, 1, :],
                                             invb[1][:])
                        nc.sync.dma_start(a2a_in[j, DH:ICB, :], an[1][:])
                    else:
                        # early-release form: drain pv out of PSUM first
                        # (raw copy + den copy) so the single pv buffer frees
                        # in ~2.7us, then normalize from SBUF off the
                        # critical path. Keeps psV at 1 buffer -> 3rd sc
                        # buffer -> scores never stall on exp completions.
                        den = invp.tile([1, HPC, QC], F32, tag="den",
                                        name=f"den{j}")
                        raw = invp.tile([DH, HPC, QC], F32, tag="raw",
                                        name=f"raw{j}")
                        inv = invp.tile([1, HPC, QC], F32, tag="inv",
                                        name=f"inv{j}")
                        invb = invp.tile([DH, HPC, QC], F32, tag="invb",
                                         name=f"invb{j}")
                        an = attp.tile([DH, HPC, QC], BF16, tag="an",
                                       name=f"an{j}")
                        nc.vector.tensor_copy(den[:], pv[j][DH:DH + 1, :, :])
                        nc.vector.tensor_copy(raw[:], pv[j][0:DH, :, :])
                        nc.vector.reciprocal_approx_fast(inv[:], den[:])
                        for h in range(HPC):
                            nc.gpsimd.partition_broadcast(invb[:, h, :],
                                                          inv[:, h, :])
                        nc.vector.tensor_mul(an[:], raw[:], invb[:])
                        load["dve"] += EPI_DVE_NS + 1400.0
                        for h in range(HPC):
                            nc.sync.dma_start(a2a_in[j, h * DH:(h + 1) * DH, :],
                                              an[:, h, :])
                    del pv[j]
                    if j == NQ - 2:
                        # re-sync cores while the last q-chunk computes
                        bar2_i = dramp.tile([1, 16], F32, tag="bar2_i")
                        bar2_o = dramp.tile([1, 16], F32, tag="bar2_o",
                                            addr_space="Shared")
                        nc.gpsimd.dma_start(bar2_i[:], a2a_in[j, 0:1, 0:16])
                        nc.gpsimd.collective_compute(
                            "AllReduce", mybir.AluOpType.add,
                            replica_groups=[list(range(NCORES))],
                            ins=[bar2_i.opt()], outs=[bar2_o.opt()],
                        )

                # macro loop: 2 score-groups, 2 exps, then a lagged PV macro
                for m in range(NM):
                    sc0 = emit_scores(2 * m)
                    sc1 = emit_scores(2 * m + 1)
                    emit_exp(m, sc0, sc1)
                    if m - LAGM >= 0:
                        emit_pv(m - LAGM)
                for m in range(NM - LAGM, NM):
                    emit_pv(m)

                # keep the PE warm through the pool transition + AllToAll
                # wait: dep-free dummies INSIDE this pool scope start the
                # instant the last PV drains (no pool-close DRAIN gap), so
                # HAM never re-throttles before the output projection
                sc_w = psS.tile([128, HPC, QC], F32, tag="sc", name="sc_warm")
                for _ in range(150):
                    nc.tensor.matmul(sc_w[:, 0, :], wz[:, 0:128], wz[:],
                                     start=True, stop=True)

            # ---- exchange: my (2 heads x all seq) -> (all inner x my seq) ----
            nc.gpsimd.collective_compute(
                "AllToAll", mybir.AluOpType.bypass,
                replica_groups=[list(range(NCORES))],
                ins=[a2a_in.opt()], outs=[a2a_out.opt()],
            )

            # ---- output projection for my SEQC rows ----
            with (
                tc.tile_pool(name="psC", bufs=2, space="PSUM") as psC,
                tc.tile_pool(name="afp", bufs=1) as afp,
                tc.tile_pool(name="finp", bufs=2) as finp,
            ):
                af = afp.tile([128, NCORES, QC], BF16, tag="af")
                for r in range(NCORES):
                    (nc.sync if r % 2 == 0 else nc.scalar).dma_start(
                        af[:, r, :], a2a_out[r])
                bo3 = bo_t[:].rearrange("p (a b) -> p a b", a=2)
                for s in range(SEQC // 128):
                    yps = psC.tile([128, 2, QC], F32, tag="y")
                    for r in range(NCORES):
                        for half in range(2):
                            nc.tensor.matmul(
                                yps[:, half, :],
                                af[:, r, s * 128:(s + 1) * 128],
                                wo_t[:, r, half * QC:(half + 1) * QC],
                                start=(r == 0), stop=(r == NCORES - 1))
                    ysb = finp.tile([128, 2, QC], F32, tag="ysb")
                    nc.vector.tensor_add(ysb[:], yps[:], bo3)
                    orows = out[s * 128:(s + 1) * 128, :].rearrange(
                        "p (a b) -> p a b", a=2)
                    for half in range(2):
                        (nc.sync if half == 0 else nc.scalar).dma_start(
                            orows[:, half, :], ysb[:, half, :])

    nc.compile()
    return nc


_NC_CACHE = None


def _get_nc():
    global _NC_CACHE
    if _NC_CACHE is None:
        _NC_CACHE = build_kernel()
    return _NC_CACHE


def _prep_inputs(x, Wq, Wk, Wv, Wo, bo):
    """Host-side sharding/layout prep (untimed)."""
    # [128 dim-rows, piece, dim-chunk, 512 seq-cols]
    xt_p = np.ascontiguousarray(
        x.T.reshape(DCH, 128, NQ, QC).transpose(1, 2, 0, 3)).astype(BF16_NP)
    wo_p = np.ascontiguousarray(
        Wo.reshape(DCH, 128, DIM).transpose(1, 0, 2)).astype(BF16_NP)
    bo_p = np.ascontiguousarray(np.tile(bo[None, :], (128, 1))).astype(np.float32)
    in_maps = []
    for c in range(NCORES):
        ic = slice(c * ICB, (c + 1) * ICB)
        m = {"xt": xt_p, "wo": wo_p, "bo": bo_p}
        for name, W in (("wq", Wq), ("wk", Wk), ("wv", Wv)):
            m[name] = np.ascontiguousarray(
                W[:, ic].reshape(DCH, 128, ICB).transpose(1, 0, 2)).astype(BF16_NP)
        in_maps.append(m)
    return in_maps


def kernel(x, Wq, Wk, Wv, Wo, bo, _trace=False):
    x = np.asarray(x, np.float32)
    Wq = np.asarray(Wq, np.float32)
    Wk = np.asarray(Wk, np.float32)
    Wv = np.asarray(Wv, np.float32)
    Wo = np.asarray(Wo, np.float32)
    bo = np.asarray(bo, np.float32)
    nc = _get_nc()
    in_maps = _prep_inputs(x, Wq, Wk, Wv, Wo, bo)
    r = run_bass_kernel_spmd(nc, in_maps, core_ids=list(range(NCORES)),
                             trace=_trace)
    y = np.concatenate([r.results[c]["out"] for c in range(NCORES)], axis=0)
    if _trace:
        kernel.last_result = r
    return y.astype(np.float32)
